# revision 8
# baseline (speedup 1.0000x reference)
"""GGNN MethodEncoder on 8 Trainium2 NeuronCores.

Strategy (no data-dependent DMA — indirect DMA is ~90us/call here):
- Nodes padded 30000->30720, sharded 3840/core (src-sharded 2D).
- Aggregation agg = A.T @ m done as dense-blocked matmul with the edge-count
  matrix uploaded as bf16 (counts are small ints, exact in bf16):
  per core, partial_agg.T = m_local.T @ A_local over local srcs, summed
  across cores via ReduceScatter (each rank keeps its dst slice).
- All activations feature-major [feat x nodes]; per-feature biases are
  per-partition scalars on the scalar engine.
- LayerNorm feature-major via ones-vector matmuls + K=1 broadcast matmuls,
  fully windowed. Mean-pool via per-tile one-hot matmuls + AllReduce.
"""
import sys

sys.path.insert(0, "/opt/trn_rl_repo")
sys.path.insert(0, "/opt/pypackages")

import numpy as np
import ml_dtypes

import concourse.bass as bass
import concourse.bacc as bacc
import concourse.mybir as mybir
from concourse import tile, masks
from concourse.bass_utils import run_bass_kernel_spmd

bf16 = mybir.dt.bfloat16
f32 = mybir.dt.float32
AF = mybir.ActivationFunctionType

NCORES = 8
N_NODES = 30000
N_PAD = 30720            # 240 tiles of 128
NLOC = N_PAD // NCORES   # 3840 per core
N_GRAPHS = 64
IN_DIM = 384
HID = 256
STEPS = 5
LN_EPS = 1e-5

W = 480                  # dst window width
NW_G = N_PAD // W        # 64 global dst windows
NW_L = NLOC // W         # 8 local windows
NT_L = NLOC // 128       # 30 local node tiles
KH = HID // 128          # 2 feature chunks


def _ln_fm(nc, work, ps, ones_col, ones_row, h_sl, gam, bet):
    """In-place LayerNorm over features; h_sl = list of KH APs [128 x NLOC]
    bf16 (feature-major). Windowed: everything per 480-node window."""
    for nw in range(NW_L):
        sl = slice(nw * W, (nw + 1) * W)
        sq = [work.tile([128, W], f32, tag="ln_sq", name="ln_sq") for _ in range(KH)]
        for k in range(KH):
            nc.vector.tensor_mul(sq[k][:], h_sl[k][:, sl], h_sl[k][:, sl])
        p1 = ps.tile([1, W], f32, tag="ps", name="ps")
        p2 = ps.tile([1, W], f32, tag="ps", name="ps")
        for k in range(KH):
            nc.tensor.matmul(p1[:], ones_col[:], h_sl[k][:, sl],
                             start=(k == 0), stop=(k == KH - 1))
        for k in range(KH):
            nc.tensor.matmul(p2[:], ones_col[:], sq[k][:],
                             start=(k == 0), stop=(k == KH - 1))
        mu = work.tile([1, W], f32, tag="ln_mu", name="ln_mu")
        var = work.tile([1, W], f32, tag="ln_var", name="ln_var")
        nc.scalar.mul(mu[:], p1[:], 1.0 / HID)
        nc.scalar.mul(var[:], p2[:], 1.0 / HID)
        musq = work.tile([1, W], f32, tag="ln_musq", name="ln_musq")
        nc.vector.tensor_mul(musq[:], mu[:], mu[:])
        nc.vector.tensor_sub(var[:], var[:], musq[:])
        nc.vector.tensor_scalar_add(var[:], var[:], float(LN_EPS))
        std = work.tile([1, W], f32, tag="ln_std", name="ln_std")
        nc.scalar.activation(std[:], var[:], AF.Sqrt, bias=0.0, scale=1.0)
        inv = work.tile([1, W], f32, tag="ln_inv", name="ln_inv")
        nc.vector.reciprocal(inv[:], std[:])
        mu_bf = work.tile([1, W], f32, tag="ln_mubf", name="ln_mubf")
        inv_bf = work.tile([1, W], f32, tag="ln_invbf", name="ln_invbf")
        nc.vector.tensor_copy(mu_bf[:], mu[:])
        nc.vector.tensor_copy(inv_bf[:], inv[:])
        bmu_ps = ps.tile([128, W], f32, tag="ps", name="ps")
        binv_ps = ps.tile([128, W], f32, tag="ps", name="ps")
        nc.tensor.matmul(bmu_ps[:], ones_row[:], mu_bf[:], start=True, stop=True)
        nc.tensor.matmul(binv_ps[:], ones_row[:], inv_bf[:], start=True, stop=True)
        bmu = work.tile([128, W], f32, tag="ln_bmu", name="ln_bmu")
        binv = work.tile([128, W], f32, tag="ln_binv", name="ln_binv")
        nc.scalar.copy(bmu[:], bmu_ps[:])
        nc.scalar.copy(binv[:], binv_ps[:])
        for k in range(KH):
            xc = work.tile([128, W], f32, tag="ln_xc", name="ln_xc")
            nc.vector.tensor_sub(xc[:], h_sl[k][:, sl], bmu[:])
            nc.vector.tensor_mul(xc[:], xc[:], binv[:])
            nc.scalar.activation(h_sl[k][:, sl], xc[:], AF.Identity,
                                 bias=bet[:, k:k + 1], scale=gam[:, k:k + 1])


def build_kernel():
    nc = bacc.Bacc("TRN2", target_bir_lowering=False, debug=False,
                   num_devices=NCORES)

    # ---- external inputs (per core) ----
    x_fm_in = nc.dram_tensor("x_fm", [IN_DIM, NLOC], bf16, kind="ExternalInput")
    a_in = nc.dram_tensor("a_cnt", [NLOC, N_PAD], bf16, kind="ExternalInput")
    lin_wT_in = nc.dram_tensor("lin_wT", [IN_DIM, HID], bf16, kind="ExternalInput")
    wg_in = nc.dram_tensor("wg", [STEPS, HID, HID], f32, kind="ExternalInput")
    w_ihT_in = nc.dram_tensor("w_ihT", [HID, 3 * HID], f32, kind="ExternalInput")
    w_hhT_in = nc.dram_tensor("w_hhT", [HID, 3 * HID], f32, kind="ExternalInput")
    lin_b_in = nc.dram_tensor("lin_b", [KH, 128, 1], f32, kind="ExternalInput")
    brz_in = nc.dram_tensor("brz", [4, 128, 1], f32, kind="ExternalInput")
    bihn_in = nc.dram_tensor("bihn", [KH, 128, 1], f32, kind="ExternalInput")
    bhhn_in = nc.dram_tensor("bhhn", [KH, 128, 1], f32, kind="ExternalInput")
    gam_in = nc.dram_tensor("gam", [KH, 128, 1], f32, kind="ExternalInput")
    bet_in = nc.dram_tensor("bet", [KH, 128, 1], f32, kind="ExternalInput")
    pool_oh_in = nc.dram_tensor("pool_oh", [NT_L, 128, N_GRAPHS], f32,
                                kind="ExternalInput")
    invcnt_in = nc.dram_tensor("invcnt", [N_GRAPHS, 1], f32, kind="ExternalInput")

    out_ext = nc.dram_tensor("out", [N_GRAPHS, HID], f32, kind="ExternalOutput")

    # ---- internal DRAM ----
    part_dram = nc.dram_tensor("part", [NW_G, KH, 128, W], f32)
    rs_out = nc.dram_tensor("rs_out", [NW_L, KH, 128, W], f32)
    pool_part = nc.dram_tensor("pool_part", [N_GRAPHS, HID], f32)
    pool_full = nc.dram_tensor("pool_full", [N_GRAPHS, HID], f32,
                               addr_space="Shared")

    rg = [list(range(NCORES))]

    with tile.TileContext(nc) as tc:
        with (
            tc.tile_pool(name="const", bufs=1) as cst,
            tc.tile_pool(name="hbuf", bufs=1) as hbuf,
            tc.tile_pool(name="abuf", bufs=3) as abuf,
            tc.tile_pool(name="xbuf", bufs=2) as xbuf,
            tc.tile_pool(name="work", bufs=2) as work,
            tc.tile_pool(name="ps", bufs=8, space="PSUM") as ps,
        ):
            # ---- constants ----
            ident = cst.tile([128, 128], f32)
            masks.make_identity(nc, ident[:])
            ones_col = cst.tile([128, 1], f32)
            nc.vector.memset(ones_col[:], 1.0)
            ones_row = cst.tile([1, 128], f32)
            nc.vector.memset(ones_row[:], 1.0)

            lin_wT = cst.tile([128, 3 * HID], bf16)
            for k in range(3):
                nc.sync.dma_start(lin_wT[:, k * HID:(k + 1) * HID],
                                  lin_wT_in[k * 128:(k + 1) * 128, :])
            wg = cst.tile([128, STEPS * KH * HID], f32)
            for i in range(STEPS):
                for k in range(KH):
                    nc.sync.dma_start(
                        wg[:, (i * KH + k) * HID:(i * KH + k + 1) * HID],
                        wg_in[i, k * 128:(k + 1) * 128, :])
            w_ihT = cst.tile([128, KH * 3 * HID], f32)
            w_hhT = cst.tile([128, KH * 3 * HID], f32)
            for k in range(KH):
                nc.sync.dma_start(w_ihT[:, k * 3 * HID:(k + 1) * 3 * HID],
                                  w_ihT_in[k * 128:(k + 1) * 128, :])
                nc.sync.dma_start(w_hhT[:, k * 3 * HID:(k + 1) * 3 * HID],
                                  w_hhT_in[k * 128:(k + 1) * 128, :])

            def load_scal(t_in, n, name):
                t = cst.tile([128, n], f32, tag=name)
                for j in range(n):
                    nc.sync.dma_start(t[:, j:j + 1], t_in[j])
                return t

            lin_b = load_scal(lin_b_in, KH, "lin_b")
            brz = load_scal(brz_in, 4, "brz")
            bihn = load_scal(bihn_in, KH, "bihn")
            bhhn = load_scal(bhhn_in, KH, "bhhn")
            gam = load_scal(gam_in, KH, "gam")
            bet = load_scal(bet_in, KH, "bet")
            invcnt = cst.tile([N_GRAPHS, 1], f32)
            nc.sync.dma_start(invcnt[:], invcnt_in[:])
            pool_oh = cst.tile([128, NT_L * N_GRAPHS], f32)
            for t in range(NT_L):
                nc.sync.dma_start(
                    pool_oh[:, t * N_GRAPHS:(t + 1) * N_GRAPHS], pool_oh_in[t])

            # ---- persistent state ----
            h_fm = hbuf.tile([128, KH * NLOC], f32)
            h_sl = [h_fm[:, k * NLOC:(k + 1) * NLOC] for k in range(KH)]
            m_sb = hbuf.tile([128, NT_L * HID], bf16)
            agg_sb = hbuf.tile([128, NW_L * KH * W], f32)

            # ---- input projection + relu ----
            for nw in range(NW_L):
                sl = slice(nw * W, (nw + 1) * W)
                xw = []
                for k in range(3):
                    xt = xbuf.tile([128, W], bf16, tag="x", name="x")
                    nc.sync.dma_start(xt[:], x_fm_in[k * 128:(k + 1) * 128, sl])
                    xw.append(xt)
                for g in range(KH):
                    pp = ps.tile([128, W], f32, tag="ps", name="ps")
                    for k in range(3):
                        nc.tensor.matmul(
                            pp[:],
                            lin_wT[:, k * HID + g * 128:k * HID + (g + 1) * 128],
                            xw[k][:],
                            start=(k == 0), stop=(k == 2))
                    nc.scalar.activation(h_sl[g][:, sl], pp[:], AF.Relu,
                                         bias=lin_b[:, g:g + 1], scale=1.0)
            _ln_fm(nc, work, ps, ones_col, ones_row, h_sl, gam, bet)

            # ---- GGNN steps ----
            for i in range(STEPS):
                # m tiles, node-major
                for t in range(NT_L):
                    pm = ps.tile([128, HID], f32, tag="ps", name="ps")
                    for k in range(KH):
                        nc.tensor.matmul(
                            pm[:],
                            h_fm[:, k * NLOC + t * 128:k * NLOC + (t + 1) * 128],
                            wg[:, (i * KH + k) * HID:(i * KH + k + 1) * HID],
                            start=(k == 0), stop=(k == KH - 1))
                    nc.scalar.copy(m_sb[:, t * HID:(t + 1) * HID], pm[:])

                # partial aggregation over local srcs, all global dst windows
                for w in range(NW_G):
                    pf = [ps.tile([128, W], f32, tag="ps", name="ps") for _ in range(KH)]
                    for s in range(NT_L):
                        at = abuf.tile([128, W], bf16, tag="a", name="a")
                        nc.sync.dma_start(
                            at[:], a_in[s * 128:(s + 1) * 128, w * W:(w + 1) * W])
                        for k in range(KH):
                            nc.tensor.matmul(
                                pf[k][:],
                                m_sb[:, s * HID + k * 128:s * HID + (k + 1) * 128],
                                at[:],
                                start=(s == 0), stop=(s == NT_L - 1))
                    for k in range(KH):
                        ev = work.tile([128, W], f32, tag="ev", name="ev")
                        nc.scalar.copy(ev[:], pf[k][:])
                        nc.sync.dma_start(part_dram[w, k], ev[:])

                nc.gpsimd.collective_compute(
                    "ReduceScatter", mybir.AluOpType.add,
                    replica_groups=rg,
                    ins=[part_dram[:]], outs=[rs_out[:]])

                for a in range(NW_L):
                    for b in range(KH):
                        nc.sync.dma_start(
                            agg_sb[:, (a * KH + b) * W:(a * KH + b + 1) * W],
                            rs_out[a, b])

                # GRU per local window
                for nw in range(NW_L):
                    agg_k = [agg_sb[:, (nw * KH + k) * W:(nw * KH + k + 1) * W]
                             for k in range(KH)]
                    rz = [ps.tile([128, W], f32, tag="ps", name="ps") for _ in range(4)]
                    inn = [ps.tile([128, W], f32, tag="ps", name="ps") for _ in range(KH)]
                    hn = [ps.tile([128, W], f32, tag="ps", name="ps") for _ in range(KH)]
                    for g in range(6):
                        dst = rz[g] if g < 4 else inn[g - 4]
                        for k in range(KH):
                            nc.tensor.matmul(
                                dst[:],
                                w_ihT[:, k * 3 * HID + g * 128:
                                      k * 3 * HID + (g + 1) * 128],
                                agg_k[k],
                                start=(k == 0), stop=(g >= 4 and k == KH - 1))
                    for g in range(6):
                        dst = rz[g] if g < 4 else hn[g - 4]
                        for k in range(KH):
                            nc.tensor.matmul(
                                dst[:],
                                w_hhT[:, k * 3 * HID + g * 128:
                                      k * 3 * HID + (g + 1) * 128],
                                h_fm[:, k * NLOC + nw * W:k * NLOC + (nw + 1) * W],
                                start=(g >= 4 and k == 0),
                                stop=(k == KH - 1))
                    r_sb, z_sb, n_sb = [], [], []
                    for g in range(KH):
                        r_t = work.tile([128, W], f32, tag="r", name="r")
                        nc.scalar.activation(r_t[:], rz[g][:], AF.Sigmoid,
                                             bias=brz[:, g:g + 1], scale=1.0)
                        r_sb.append(r_t)
                        z_t = work.tile([128, W], f32, tag="z", name="z")
                        nc.scalar.activation(z_t[:], rz[KH + g][:], AF.Sigmoid,
                                             bias=brz[:, KH + g:KH + g + 1],
                                             scale=1.0)
                        z_sb.append(z_t)
                    for g in range(KH):
                        t1 = work.tile([128, W], f32, tag="t1", name="t1")
                        nc.scalar.activation(t1[:], hn[g][:], AF.Identity,
                                             bias=bhhn[:, g:g + 1], scale=1.0)
                        t2 = work.tile([128, W], f32, tag="t2", name="t2")
                        nc.vector.tensor_mul(t2[:], r_sb[g][:], t1[:])
                        t3 = work.tile([128, W], f32, tag="t3", name="t3")
                        nc.vector.tensor_add(t3[:], t2[:], inn[g][:])
                        n_t = work.tile([128, W], f32, tag="n", name="n")
                        nc.scalar.activation(n_t[:], t3[:], AF.Tanh,
                                             bias=bihn[:, g:g + 1], scale=1.0)
                        n_sb.append(n_t)
                    for g in range(KH):
                        hsl = h_fm[:, g * NLOC + nw * W:g * NLOC + (nw + 1) * W]
                        hmn = work.tile([128, W], f32, tag="hmn", name="hmn")
                        nc.vector.tensor_sub(hmn[:], hsl, n_sb[g][:])
                        zm = work.tile([128, W], f32, tag="zm", name="zm")
                        nc.vector.tensor_mul(zm[:], z_sb[g][:], hmn[:])
                        nc.vector.tensor_add(hsl, n_sb[g][:], zm[:])

            # ---- final LN ----
            _ln_fm(nc, work, ps, ones_col, ones_row, h_sl, gam, bet)

            # ---- pooling ----
            pool_ps = ps.tile([N_GRAPHS, HID], f32, tag="ps", name="ps")
            for t in range(NT_L):
                pnm = ps.tile([128, HID], f32, tag="ps", name="ps")
                for k in range(KH):
                    nc.tensor.matmul(
                        pnm[:, k * 128:(k + 1) * 128],
                        h_fm[:, k * NLOC + t * 128:k * NLOC + (t + 1) * 128],
                        ident[:],
                        start=(k == 0), stop=(k == KH - 1))
                h_nm = work.tile([128, HID], f32, tag="hnm", name="hnm")
                nc.scalar.copy(h_nm[:], pnm[:])
                nc.tensor.matmul(pool_ps[:],
                                 pool_oh[:, t * N_GRAPHS:(t + 1) * N_GRAPHS],
                                 h_nm[:],
                                 start=(t == 0), stop=(t == NT_L - 1))
            pool_sb = work.tile([N_GRAPHS, HID], f32, tag="pool", name="pool")
            nc.vector.tensor_copy(pool_sb[:], pool_ps[:])
            nc.sync.dma_start(pool_part[:], pool_sb[:])
            nc.gpsimd.collective_compute(
                "AllReduce", mybir.AluOpType.add, replica_groups=rg,
                ins=[pool_part[:]], outs=[pool_full[:]])
            pf_sb = work.tile([N_GRAPHS, HID], f32, tag="poolf", name="poolf")
            nc.sync.dma_start(pf_sb[:], pool_full[:])
            po_sb = work.tile([N_GRAPHS, HID], f32, tag="poolo", name="poolo")
            nc.scalar.activation(po_sb[:], pf_sb[:], AF.Copy,
                                 scale=invcnt[:], bias=0.0)
            nc.sync.dma_start(out_ext[:], po_sb[:])

    nc.compile()
    return nc


_NC_CACHE = {}


def _prep_inputs(x, edge_index, batch, lin_w, lin_b, gamma, beta,
                 ggnn_w, w_ih, w_hh, b_ih, b_hh):
    bfa = ml_dtypes.bfloat16
    x_pad = np.zeros((N_PAD, IN_DIM), np.float32)
    x_pad[:N_NODES] = np.asarray(x, np.float32)
    src = np.asarray(edge_index[0], np.int64)
    dst = np.asarray(edge_index[1], np.int64)
    batch = np.asarray(batch, np.int64)

    lin_wT = np.asarray(lin_w, np.float32).T.astype(bfa)
    wgs = np.ascontiguousarray(np.asarray(ggnn_w, np.float32))
    w_ihT = np.ascontiguousarray(np.asarray(w_ih, np.float32).T)
    w_hhT = np.ascontiguousarray(np.asarray(w_hh, np.float32).T)
    b_ih = np.asarray(b_ih, np.float32)
    b_hh = np.asarray(b_hh, np.float32)

    def chunks(v, n):
        return np.ascontiguousarray(v.reshape(n, 128, 1).astype(np.float32))

    brz = chunks((b_ih + b_hh)[:2 * HID], 4)
    bihn = chunks(b_ih[2 * HID:], KH)
    bhhn = chunks(b_hh[2 * HID:], KH)
    lin_b_c = chunks(np.asarray(lin_b, np.float32), KH)
    gam_c = chunks(np.asarray(gamma, np.float32), KH)
    bet_c = chunks(np.asarray(beta, np.float32), KH)

    counts = np.bincount(batch, minlength=N_GRAPHS).astype(np.float32)
    invcnt = (1.0 / np.maximum(counts, 1.0)).reshape(N_GRAPHS, 1).astype(np.float32)

    in_maps = []
    for c in range(NCORES):
        lo, hi = c * NLOC, (c + 1) * NLOC
        a_loc = np.zeros((NLOC, N_PAD), np.float32)
        msel = (src >= lo) & (src < hi)
        np.add.at(a_loc, (src[msel] - lo, dst[msel]), 1.0)

        pool_oh = np.zeros((NT_L, 128, N_GRAPHS), np.float32)
        node_ids = np.arange(lo, hi)
        valid = node_ids < N_NODES
        g_of = np.where(valid, batch[np.minimum(node_ids, N_NODES - 1)], 0)
        for t in range(NT_L):
            rows = np.arange(128)
            v = valid[t * 128:(t + 1) * 128]
            pool_oh[t, rows[v], g_of[t * 128:(t + 1) * 128][v]] = 1.0

        in_maps.append({
            "x_fm": np.ascontiguousarray(x_pad[lo:hi].T).astype(bfa),
            "a_cnt": a_loc.astype(bfa),
            "lin_wT": lin_wT, "wg": wgs, "w_ihT": w_ihT, "w_hhT": w_hhT,
            "lin_b": lin_b_c, "brz": brz, "bihn": bihn, "bhhn": bhhn,
            "gam": gam_c, "bet": bet_c,
            "pool_oh": np.ascontiguousarray(pool_oh),
            "invcnt": invcnt,
        })
    return in_maps


def kernel(**inputs):
    if "nc" not in _NC_CACHE:
        _NC_CACHE["nc"] = build_kernel()
    nc = _NC_CACHE["nc"]
    in_maps = _prep_inputs(**inputs)
    res = run_bass_kernel_spmd(nc, in_maps, list(range(NCORES)))
    return np.asarray(res.results[0]["out"]).astype(np.float32)


# revision 9
# speedup vs baseline: 1.2165x; 1.2165x over previous
"""GGNN MethodEncoder on 8 Trainium2 NeuronCores.

Strategy (no data-dependent DMA — indirect DMA is ~90us/call here):
- Nodes padded 30000->30720, sharded 3840/core (src-sharded 2D).
- Aggregation agg = A.T @ m done as dense-blocked matmul with the edge-count
  matrix uploaded as bf16 (counts are small ints, exact in bf16):
  per core, partial_agg.T = m_local.T @ A_local over local srcs, summed
  across cores via ReduceScatter (each rank keeps its dst slice).
- All activations feature-major [feat x nodes]; per-feature biases are
  per-partition scalars on the scalar engine.
- LayerNorm feature-major via ones-vector matmuls + K=1 broadcast matmuls,
  fully windowed. Mean-pool via per-tile one-hot matmuls + AllReduce.
"""
import sys

sys.path.insert(0, "/opt/trn_rl_repo")
sys.path.insert(0, "/opt/pypackages")

import numpy as np
import ml_dtypes

import concourse.bass as bass
import concourse.bacc as bacc
import concourse.mybir as mybir
from concourse import tile, masks
from concourse.bass_utils import run_bass_kernel_spmd

bf16 = mybir.dt.bfloat16
f32 = mybir.dt.float32
AF = mybir.ActivationFunctionType

NCORES = 8
N_NODES = 30000
N_PAD = 30720            # 240 tiles of 128
NLOC = N_PAD // NCORES   # 3840 per core
N_GRAPHS = 64
IN_DIM = 384
HID = 256
STEPS = 5
LN_EPS = 1e-5

W = 480                  # dst window width
NW_G = N_PAD // W        # 64 global dst windows
NW_L = NLOC // W         # 8 local windows
NT_L = NLOC // 128       # 30 local node tiles
KH = HID // 128          # 2 feature chunks


def _ln_fm(nc, work, ps, ones_col, ones_row, h_sl, gam, bet):
    """In-place LayerNorm over features; h_sl = list of KH APs [128 x NLOC]
    bf16 (feature-major). Windowed: everything per 480-node window."""
    for nw in range(NW_L):
        sl = slice(nw * W, (nw + 1) * W)
        sq = [work.tile([128, W], f32, tag="ln_sq", name="ln_sq") for _ in range(KH)]
        for k in range(KH):
            nc.vector.tensor_mul(sq[k][:], h_sl[k][:, sl], h_sl[k][:, sl])
        p1 = ps.tile([1, W], f32, tag="ps", name="ps")
        p2 = ps.tile([1, W], f32, tag="ps", name="ps")
        for k in range(KH):
            nc.tensor.matmul(p1[:], ones_col[:], h_sl[k][:, sl],
                             start=(k == 0), stop=(k == KH - 1))
        for k in range(KH):
            nc.tensor.matmul(p2[:], ones_col[:], sq[k][:],
                             start=(k == 0), stop=(k == KH - 1))
        mu = work.tile([1, W], f32, tag="ln_mu", name="ln_mu")
        var = work.tile([1, W], f32, tag="ln_var", name="ln_var")
        nc.scalar.mul(mu[:], p1[:], 1.0 / HID)
        nc.scalar.mul(var[:], p2[:], 1.0 / HID)
        musq = work.tile([1, W], f32, tag="ln_musq", name="ln_musq")
        nc.vector.tensor_mul(musq[:], mu[:], mu[:])
        nc.vector.tensor_sub(var[:], var[:], musq[:])
        nc.vector.tensor_scalar_add(var[:], var[:], float(LN_EPS))
        std = work.tile([1, W], f32, tag="ln_std", name="ln_std")
        nc.scalar.activation(std[:], var[:], AF.Sqrt, bias=0.0, scale=1.0)
        inv = work.tile([1, W], f32, tag="ln_inv", name="ln_inv")
        nc.vector.reciprocal(inv[:], std[:])
        mu_bf = work.tile([1, W], f32, tag="ln_mubf", name="ln_mubf")
        inv_bf = work.tile([1, W], f32, tag="ln_invbf", name="ln_invbf")
        nc.vector.tensor_copy(mu_bf[:], mu[:])
        nc.vector.tensor_copy(inv_bf[:], inv[:])
        bmu_ps = ps.tile([128, W], f32, tag="ps", name="ps")
        binv_ps = ps.tile([128, W], f32, tag="ps", name="ps")
        nc.tensor.matmul(bmu_ps[:], ones_row[:], mu_bf[:], start=True, stop=True)
        nc.tensor.matmul(binv_ps[:], ones_row[:], inv_bf[:], start=True, stop=True)
        bmu = work.tile([128, W], f32, tag="ln_bmu", name="ln_bmu")
        binv = work.tile([128, W], f32, tag="ln_binv", name="ln_binv")
        nc.scalar.copy(bmu[:], bmu_ps[:])
        nc.scalar.copy(binv[:], binv_ps[:])
        for k in range(KH):
            xc = work.tile([128, W], f32, tag="ln_xc", name="ln_xc")
            nc.vector.tensor_sub(xc[:], h_sl[k][:, sl], bmu[:])
            nc.vector.tensor_mul(xc[:], xc[:], binv[:])
            nc.scalar.activation(h_sl[k][:, sl], xc[:], AF.Identity,
                                 bias=bet[:, k:k + 1], scale=gam[:, k:k + 1])


def build_kernel():
    nc = bacc.Bacc("TRN2", target_bir_lowering=False, debug=False,
                   num_devices=NCORES)

    # ---- external inputs (per core) ----
    x_fm_in = nc.dram_tensor("x_fm", [IN_DIM, NLOC], bf16, kind="ExternalInput")
    fp8 = mybir.dt.float8e4
    a_in = nc.dram_tensor("a_cnt", [NLOC, N_PAD], fp8, kind="ExternalInput")
    lin_wT_in = nc.dram_tensor("lin_wT", [IN_DIM, HID], bf16, kind="ExternalInput")
    wg_in = nc.dram_tensor("wg", [STEPS, HID, HID], f32, kind="ExternalInput")
    w_ihT_in = nc.dram_tensor("w_ihT", [HID, 3 * HID], f32, kind="ExternalInput")
    w_hhT_in = nc.dram_tensor("w_hhT", [HID, 3 * HID], f32, kind="ExternalInput")
    lin_b_in = nc.dram_tensor("lin_b", [KH, 128, 1], f32, kind="ExternalInput")
    brz_in = nc.dram_tensor("brz", [4, 128, 1], f32, kind="ExternalInput")
    bihn_in = nc.dram_tensor("bihn", [KH, 128, 1], f32, kind="ExternalInput")
    bhhn_in = nc.dram_tensor("bhhn", [KH, 128, 1], f32, kind="ExternalInput")
    gam_in = nc.dram_tensor("gam", [KH, 128, 1], f32, kind="ExternalInput")
    bet_in = nc.dram_tensor("bet", [KH, 128, 1], f32, kind="ExternalInput")
    pool_oh_in = nc.dram_tensor("pool_oh", [NT_L, 128, N_GRAPHS], f32,
                                kind="ExternalInput")
    invcnt_in = nc.dram_tensor("invcnt", [N_GRAPHS, 1], f32, kind="ExternalInput")

    out_ext = nc.dram_tensor("out", [N_GRAPHS, HID], f32, kind="ExternalOutput")

    # ---- internal DRAM ----
    part_dram = nc.dram_tensor("part", [NW_G, KH, 128, W], f32)
    rs_out = nc.dram_tensor("rs_out", [NW_L, KH, 128, W], f32)
    pool_part = nc.dram_tensor("pool_part", [N_GRAPHS, HID], f32)
    pool_full = nc.dram_tensor("pool_full", [N_GRAPHS, HID], f32,
                               addr_space="Shared")

    rg = [list(range(NCORES))]

    with tile.TileContext(nc) as tc:
        with (
            tc.tile_pool(name="const", bufs=1) as cst,
            tc.tile_pool(name="hbuf", bufs=1) as hbuf,
            tc.tile_pool(name="abuf", bufs=4) as abuf,
            tc.tile_pool(name="xbuf", bufs=2) as xbuf,
            tc.tile_pool(name="work", bufs=2) as work,
            tc.tile_pool(name="ps", bufs=8, space="PSUM") as ps,
        ):
            # ---- constants ----
            ident = cst.tile([128, 128], f32)
            masks.make_identity(nc, ident[:])
            ones_col = cst.tile([128, 1], f32)
            nc.vector.memset(ones_col[:], 1.0)
            ones_row = cst.tile([1, 128], f32)
            nc.vector.memset(ones_row[:], 1.0)

            lin_wT = cst.tile([128, 3 * HID], bf16)
            for k in range(3):
                nc.sync.dma_start(lin_wT[:, k * HID:(k + 1) * HID],
                                  lin_wT_in[k * 128:(k + 1) * 128, :])
            wg = cst.tile([128, STEPS * KH * HID], f32)
            for i in range(STEPS):
                for k in range(KH):
                    nc.sync.dma_start(
                        wg[:, (i * KH + k) * HID:(i * KH + k + 1) * HID],
                        wg_in[i, k * 128:(k + 1) * 128, :])
            w_ihT = cst.tile([128, KH * 3 * HID], f32)
            w_hhT = cst.tile([128, KH * 3 * HID], f32)
            for k in range(KH):
                nc.sync.dma_start(w_ihT[:, k * 3 * HID:(k + 1) * 3 * HID],
                                  w_ihT_in[k * 128:(k + 1) * 128, :])
                nc.sync.dma_start(w_hhT[:, k * 3 * HID:(k + 1) * 3 * HID],
                                  w_hhT_in[k * 128:(k + 1) * 128, :])

            def load_scal(t_in, n, name):
                t = cst.tile([128, n], f32, tag=name)
                for j in range(n):
                    nc.sync.dma_start(t[:, j:j + 1], t_in[j])
                return t

            lin_b = load_scal(lin_b_in, KH, "lin_b")
            brz = load_scal(brz_in, 4, "brz")
            bihn = load_scal(bihn_in, KH, "bihn")
            bhhn = load_scal(bhhn_in, KH, "bhhn")
            gam = load_scal(gam_in, KH, "gam")
            bet = load_scal(bet_in, KH, "bet")
            invcnt = cst.tile([N_GRAPHS, 1], f32)
            nc.sync.dma_start(invcnt[:], invcnt_in[:])
            pool_oh = cst.tile([128, NT_L * N_GRAPHS], f32)
            for t in range(NT_L):
                nc.sync.dma_start(
                    pool_oh[:, t * N_GRAPHS:(t + 1) * N_GRAPHS], pool_oh_in[t])

            # ---- persistent state ----
            h_fm = hbuf.tile([128, KH * NLOC], f32)
            h_sl = [h_fm[:, k * NLOC:(k + 1) * NLOC] for k in range(KH)]
            m_sb = hbuf.tile([128, NT_L * HID], bf16)
            agg_sb = hbuf.tile([128, NW_L * KH * W], f32)

            # ---- input projection + relu ----
            for nw in range(NW_L):
                sl = slice(nw * W, (nw + 1) * W)
                xw = []
                for k in range(3):
                    xt = xbuf.tile([128, W], bf16, tag="x", name="x")
                    nc.sync.dma_start(xt[:], x_fm_in[k * 128:(k + 1) * 128, sl])
                    xw.append(xt)
                for g in range(KH):
                    pp = ps.tile([128, W], f32, tag="ps", name="ps")
                    for k in range(3):
                        nc.tensor.matmul(
                            pp[:],
                            lin_wT[:, k * HID + g * 128:k * HID + (g + 1) * 128],
                            xw[k][:],
                            start=(k == 0), stop=(k == 2))
                    nc.scalar.activation(h_sl[g][:, sl], pp[:], AF.Relu,
                                         bias=lin_b[:, g:g + 1], scale=1.0)
            _ln_fm(nc, work, ps, ones_col, ones_row, h_sl, gam, bet)

            # ---- GGNN steps ----
            for i in range(STEPS):
                # m tiles, node-major
                for t in range(NT_L):
                    pm = ps.tile([128, HID], f32, tag="ps", name="ps")
                    for k in range(KH):
                        nc.tensor.matmul(
                            pm[:],
                            h_fm[:, k * NLOC + t * 128:k * NLOC + (t + 1) * 128],
                            wg[:, (i * KH + k) * HID:(i * KH + k + 1) * HID],
                            start=(k == 0), stop=(k == KH - 1))
                    nc.scalar.copy(m_sb[:, t * HID:(t + 1) * HID], pm[:])

                # partial aggregation over local srcs, all global dst windows
                for w in range(NW_G):
                    pf = [ps.tile([128, W], f32, tag="ps", name="ps") for _ in range(KH)]
                    for s in range(NT_L):
                        at = abuf.tile([128, W], fp8, tag="a", name="a")
                        nc.sync.dma_start(
                            at[:], a_in[s * 128:(s + 1) * 128, w * W:(w + 1) * W])
                        for k in range(KH):
                            nc.tensor.matmul(
                                pf[k][:],
                                m_sb[:, s * HID + k * 128:s * HID + (k + 1) * 128],
                                at[:],
                                start=(s == 0), stop=(s == NT_L - 1))
                    for k in range(KH):
                        ev = work.tile([128, W], f32, tag="ev", name="ev")
                        nc.scalar.copy(ev[:], pf[k][:])
                        nc.sync.dma_start(part_dram[w, k], ev[:])

                nc.gpsimd.collective_compute(
                    "ReduceScatter", mybir.AluOpType.add,
                    replica_groups=rg,
                    ins=[part_dram[:]], outs=[rs_out[:]])

                for a in range(NW_L):
                    for b in range(KH):
                        nc.sync.dma_start(
                            agg_sb[:, (a * KH + b) * W:(a * KH + b + 1) * W],
                            rs_out[a, b])

                # GRU per local window
                for nw in range(NW_L):
                    agg_k = [agg_sb[:, (nw * KH + k) * W:(nw * KH + k + 1) * W]
                             for k in range(KH)]
                    rz = [ps.tile([128, W], f32, tag="ps", name="ps") for _ in range(4)]
                    inn = [ps.tile([128, W], f32, tag="ps", name="ps") for _ in range(KH)]
                    hn = [ps.tile([128, W], f32, tag="ps", name="ps") for _ in range(KH)]
                    for g in range(6):
                        dst = rz[g] if g < 4 else inn[g - 4]
                        for k in range(KH):
                            nc.tensor.matmul(
                                dst[:],
                                w_ihT[:, k * 3 * HID + g * 128:
                                      k * 3 * HID + (g + 1) * 128],
                                agg_k[k],
                                start=(k == 0), stop=(g >= 4 and k == KH - 1))
                    for g in range(6):
                        dst = rz[g] if g < 4 else hn[g - 4]
                        for k in range(KH):
                            nc.tensor.matmul(
                                dst[:],
                                w_hhT[:, k * 3 * HID + g * 128:
                                      k * 3 * HID + (g + 1) * 128],
                                h_fm[:, k * NLOC + nw * W:k * NLOC + (nw + 1) * W],
                                start=(g >= 4 and k == 0),
                                stop=(k == KH - 1))
                    r_sb, z_sb, n_sb = [], [], []
                    for g in range(KH):
                        r_t = work.tile([128, W], f32, tag="r", name="r")
                        nc.scalar.activation(r_t[:], rz[g][:], AF.Sigmoid,
                                             bias=brz[:, g:g + 1], scale=1.0)
                        r_sb.append(r_t)
                        z_t = work.tile([128, W], f32, tag="z", name="z")
                        nc.scalar.activation(z_t[:], rz[KH + g][:], AF.Sigmoid,
                                             bias=brz[:, KH + g:KH + g + 1],
                                             scale=1.0)
                        z_sb.append(z_t)
                    for g in range(KH):
                        t1 = work.tile([128, W], f32, tag="t1", name="t1")
                        nc.scalar.activation(t1[:], hn[g][:], AF.Identity,
                                             bias=bhhn[:, g:g + 1], scale=1.0)
                        t2 = work.tile([128, W], f32, tag="t2", name="t2")
                        nc.vector.tensor_mul(t2[:], r_sb[g][:], t1[:])
                        t3 = work.tile([128, W], f32, tag="t3", name="t3")
                        nc.vector.tensor_add(t3[:], t2[:], inn[g][:])
                        n_t = work.tile([128, W], f32, tag="n", name="n")
                        nc.scalar.activation(n_t[:], t3[:], AF.Tanh,
                                             bias=bihn[:, g:g + 1], scale=1.0)
                        n_sb.append(n_t)
                    for g in range(KH):
                        hsl = h_fm[:, g * NLOC + nw * W:g * NLOC + (nw + 1) * W]
                        hmn = work.tile([128, W], f32, tag="hmn", name="hmn")
                        nc.vector.tensor_sub(hmn[:], hsl, n_sb[g][:])
                        zm = work.tile([128, W], f32, tag="zm", name="zm")
                        nc.vector.tensor_mul(zm[:], z_sb[g][:], hmn[:])
                        nc.vector.tensor_add(hsl, n_sb[g][:], zm[:])

            # ---- final LN ----
            _ln_fm(nc, work, ps, ones_col, ones_row, h_sl, gam, bet)

            # ---- pooling ----
            pool_ps = ps.tile([N_GRAPHS, HID], f32, tag="ps", name="ps")
            for t in range(NT_L):
                pnm = ps.tile([128, HID], f32, tag="ps", name="ps")
                for k in range(KH):
                    nc.tensor.matmul(
                        pnm[:, k * 128:(k + 1) * 128],
                        h_fm[:, k * NLOC + t * 128:k * NLOC + (t + 1) * 128],
                        ident[:],
                        start=(k == 0), stop=(k == KH - 1))
                h_nm = work.tile([128, HID], f32, tag="hnm", name="hnm")
                nc.scalar.copy(h_nm[:], pnm[:])
                nc.tensor.matmul(pool_ps[:],
                                 pool_oh[:, t * N_GRAPHS:(t + 1) * N_GRAPHS],
                                 h_nm[:],
                                 start=(t == 0), stop=(t == NT_L - 1))
            pool_sb = work.tile([N_GRAPHS, HID], f32, tag="pool", name="pool")
            nc.vector.tensor_copy(pool_sb[:], pool_ps[:])
            nc.sync.dma_start(pool_part[:], pool_sb[:])
            nc.gpsimd.collective_compute(
                "AllReduce", mybir.AluOpType.add, replica_groups=rg,
                ins=[pool_part[:]], outs=[pool_full[:]])
            pf_sb = work.tile([N_GRAPHS, HID], f32, tag="poolf", name="poolf")
            nc.sync.dma_start(pf_sb[:], pool_full[:])
            po_sb = work.tile([N_GRAPHS, HID], f32, tag="poolo", name="poolo")
            nc.scalar.activation(po_sb[:], pf_sb[:], AF.Copy,
                                 scale=invcnt[:], bias=0.0)
            nc.sync.dma_start(out_ext[:], po_sb[:])

    nc.compile()
    return nc


_NC_CACHE = {}


def _prep_inputs(x, edge_index, batch, lin_w, lin_b, gamma, beta,
                 ggnn_w, w_ih, w_hh, b_ih, b_hh):
    bfa = ml_dtypes.bfloat16
    x_pad = np.zeros((N_PAD, IN_DIM), np.float32)
    x_pad[:N_NODES] = np.asarray(x, np.float32)
    src = np.asarray(edge_index[0], np.int64)
    dst = np.asarray(edge_index[1], np.int64)
    batch = np.asarray(batch, np.int64)

    lin_wT = np.asarray(lin_w, np.float32).T.astype(bfa)
    wgs = np.ascontiguousarray(np.asarray(ggnn_w, np.float32))
    w_ihT = np.ascontiguousarray(np.asarray(w_ih, np.float32).T)
    w_hhT = np.ascontiguousarray(np.asarray(w_hh, np.float32).T)
    b_ih = np.asarray(b_ih, np.float32)
    b_hh = np.asarray(b_hh, np.float32)

    def chunks(v, n):
        return np.ascontiguousarray(v.reshape(n, 128, 1).astype(np.float32))

    brz = chunks((b_ih + b_hh)[:2 * HID], 4)
    bihn = chunks(b_ih[2 * HID:], KH)
    bhhn = chunks(b_hh[2 * HID:], KH)
    lin_b_c = chunks(np.asarray(lin_b, np.float32), KH)
    gam_c = chunks(np.asarray(gamma, np.float32), KH)
    bet_c = chunks(np.asarray(beta, np.float32), KH)

    counts = np.bincount(batch, minlength=N_GRAPHS).astype(np.float32)
    invcnt = (1.0 / np.maximum(counts, 1.0)).reshape(N_GRAPHS, 1).astype(np.float32)

    in_maps = []
    for c in range(NCORES):
        lo, hi = c * NLOC, (c + 1) * NLOC
        a_loc = np.zeros((NLOC, N_PAD), np.float32)
        msel = (src >= lo) & (src < hi)
        np.add.at(a_loc, (src[msel] - lo, dst[msel]), 1.0)

        pool_oh = np.zeros((NT_L, 128, N_GRAPHS), np.float32)
        node_ids = np.arange(lo, hi)
        valid = node_ids < N_NODES
        g_of = np.where(valid, batch[np.minimum(node_ids, N_NODES - 1)], 0)
        for t in range(NT_L):
            rows = np.arange(128)
            v = valid[t * 128:(t + 1) * 128]
            pool_oh[t, rows[v], g_of[t * 128:(t + 1) * 128][v]] = 1.0

        in_maps.append({
            "x_fm": np.ascontiguousarray(x_pad[lo:hi].T).astype(bfa),
            "a_cnt": a_loc.astype(ml_dtypes.float8_e4m3fn),
            "lin_wT": lin_wT, "wg": wgs, "w_ihT": w_ihT, "w_hhT": w_hhT,
            "lin_b": lin_b_c, "brz": brz, "bihn": bihn, "bhhn": bhhn,
            "gam": gam_c, "bet": bet_c,
            "pool_oh": np.ascontiguousarray(pool_oh),
            "invcnt": invcnt,
        })
    return in_maps


def kernel(**inputs):
    if "nc" not in _NC_CACHE:
        _NC_CACHE["nc"] = build_kernel()
    nc = _NC_CACHE["nc"]
    in_maps = _prep_inputs(**inputs)
    res = run_bass_kernel_spmd(nc, in_maps, list(range(NCORES)))
    return np.asarray(res.results[0]["out"]).astype(np.float32)
